# revision 30
# baseline (speedup 1.0000x reference)
"""LSNN layer forward on 8 Trainium2 NeuronCores (data-parallel over batch).

Reference math (per batch row):
    L1    = x_t @ W_syn.T + b_syn
    alpha = sigmoid((L1 + u_t) @ W_Tm.T + b_Tm)
    rho   = sigmoid((L1 + b_t) @ W_Tadp.T + b_Tadp)
    b_new = rho * b_t + (1 - rho) * spk
    thr   = 0.01 + 1.8 * b_new
    u_new = u_t + (L1 - u_t) / alpha
    o_spk = (u_new - thr > 0) as f32

Fast path (used when u_t == b_t == spk == 0, which the runtime check in
kernel() verifies): the rho branch cannot affect the output
(b_new = rho*0 + (1-rho)*0 = 0, thr = 0.01) and u_new = L1 / alpha with
alpha = sigmoid(L1 @ W_Tm.T + b_Tm) > 0, so

    o_spk = (L1 / alpha > 0.01) = (100 * L1 > sigmoid(z1 + b_Tm))

Only two device matmuls remain:
  phase 1: L1 = W_syn @ x in f32r (precision-critical; 1 PE col/cycle)
  phase 2: z1 = W_Tm @ L1 in float8e4 DoubleRow (weights x64, L1 x8 to
           stay in the fp8 normal range; 2 k-tiles per matmul = 2x PE
           throughput; the sigmoid rescales by 1/512 for free). The
           sigmoid path is heavily damped: fp8 moves only ~40 spikes
           out of 8.4M beyond the f32r-mm1 baseline of ~322.
Per-tile tail is one ACT sigmoid + one DVE is_gt compare (u8 out); only
the sigmoid table set is loaded (the baseline's per-tile Exp/Sigmoid
alternation cost 32 ACT table loads = 41us).
Dummy warm-up matmuls fire the PE HAM clock gate (1.2 -> 2.4 GHz) while
the x DMA head is still landing.

General path (any nonzero state): the original 3-matmul kernel, kept
verbatim below.

Sharding: batch 4096 -> 8 shards of 512; weights replicated; no
cross-core communication.

Measured on trn2 (8 cores, per-core 512x2048 batch shard):
    original baseline    227.4us, rel err 8.78e-3 (322/8.4M spike flips)
    fast bf16 mm2        145.0us, rel err 8.9e-3
    fast fp8-DR mm2      116.6us, rel err 9.34e-3 (364 flips)
Engine budget of the fast path per core: ~8.5us NEFF preamble/barriers,
~16us x+w0 DMA head (PE warms up + runs tile 0), ~52us phase-1 (256
f32r MMs, DMA stays ahead), ~27.6us phase-2 (128 fp8-DR MMs), ~5us
sigmoid/compare/output tail + postamble. DMA total 26MB at ~350-420GB/s
effective; PE is the bottleneck (target_regime=compute).
"""

import os

import numpy as np
import ml_dtypes

import concourse.bacc as bacc
import concourse.tile as tile
import concourse.mybir as mybir
from concourse.bass_utils import run_bass_kernel_spmd

AF = mybir.ActivationFunctionType
ALU = mybir.AluOpType

B, I, O = 4096, 2048, 2048
NCORES = 8
BC = B // NCORES          # 512 batch rows per core
P = 128                   # partitions
KT = I // P               # 16 k-tiles
OT = O // P               # 16 output neuron tiles
THR_MIN = 0.01

F32 = mybir.dt.float32
F32R = mybir.dt.float32r
BF16 = mybir.dt.bfloat16
FP8 = mybir.dt.float8e4
U8 = mybir.dt.uint8

# fast-path mm2 dtype: "fp8" uses float8e4 weights (x64) + activations
# (x8) with DoubleRow (2 k-tiles per matmul, 2x tensor-engine
# throughput); "bf16" casts L1 to bf16 (both operands bf16).
# ("mixed" bf16 x f32r is rejected by the BIR verifier: fp32/f32r operands
# must have matching transfer types.)
FAST_MM2 = os.environ.get("FAST_MM2", "fp8")
W2_SCALE = 64.0   # fp8 weight pre-scale (power of 2, exact)
A2_SCALE = 8.0    # fp8 activation pre-scale


def build_nc_fast():
    fp8 = FAST_MM2 == "fp8"
    rhs_dt = FP8 if fp8 else (F32R if FAST_MM2 == "mixed" else BF16)
    w2_dt = FP8 if fp8 else BF16
    # ACT scale folding: l1r holds A2_SCALE*L1 in fp8 mode; the sigmoid
    # rescales psum by 1/(W2_SCALE*A2_SCALE).
    a_scale = A2_SCALE if fp8 else 1.0
    sig_scale = 1.0 / (W2_SCALE * A2_SCALE) if fp8 else 1.0

    nc = bacc.Bacc("TRN2", target_bir_lowering=False, debug=False)

    xh_d = nc.dram_tensor("xh", (P, KT, BC), F32R, kind="ExternalInput").ap()
    wsyn_d = nc.dram_tensor("wsyn", (P, OT, KT, P), F32R, kind="ExternalInput").ap()
    wtm_d = nc.dram_tensor("wtm", (P, OT, KT, P), w2_dt, kind="ExternalInput").ap()
    bsynr_d = nc.dram_tensor("bsynr", (P, OT), F32, kind="ExternalInput").ap()
    bs100_d = nc.dram_tensor("bs100", (P, OT), F32, kind="ExternalInput").ap()
    btm_d = nc.dram_tensor("btm", (P, OT), F32, kind="ExternalInput").ap()
    out_d = nc.dram_tensor("out", (OT, P, BC), U8, kind="ExternalOutput").ap()

    with tile.TileContext(nc) as tc:
        with (
            tc.tile_pool(name="persist", bufs=1) as persist,
            tc.tile_pool(name="wpool", bufs=6) as wpool,
            tc.tile_pool(name="tmp", bufs=4) as tmp,
            tc.tile_pool(name="outp", bufs=3) as outp,
            tc.tile_pool(name="psum1", bufs=2, space="PSUM") as psum1,
            tc.tile_pool(name="psum2", bufs=5, space="PSUM") as psum2,
            tc.tile_pool(name="psumw", bufs=1, space="PSUM") as psumw,
        ):
            xsb = persist.tile([P, KT, BC], F32R, tag="xsb")
            l1r = persist.tile([P, OT, BC], rhs_dt, tag="l1r")   # a_scale*L1
            l1s = persist.tile([P, OT, BC], F32, tag="l1s")      # 100*L1
            bsynr = persist.tile([P, OT], F32, tag="bsynr")
            bs100 = persist.tile([P, OT], F32, tag="bs100")
            btm = persist.tile([P, OT], F32, tag="btm")

            # PE warm-up: the HAM clock gate keeps the PE at 1.2 GHz until
            # it has seen ~3.4us of sustained matmul activity. The input
            # DMA head (x + first weight tiles) takes ~12us, during which
            # the PE would otherwise idle/dribble and stay cold. Dummy
            # matmuls on zeroed tiles into a scratch PSUM bank warm the
            # clock to 2.4 GHz before the real matmuls begin.
            wu_w = persist.tile([P, P], BF16, tag="wu_w")
            wu_x = persist.tile([P, BC], BF16, tag="wu_x")
            nc.vector.memset(wu_w[:], 0.0)
            nc.vector.memset(wu_x[:], 0.0)
            wups = psumw.tile([P, BC], F32, tag="wu_ps")

            def filler(n):
                # Dummy matmuls into the scratch PSUM bank. Interleaved
                # into the DMA-gated head they keep the PE's HAM activity
                # window busy, so the clock never drops back to 1.2 GHz
                # while x/weight chunks are still landing.
                for _ in range(n):
                    nc.tensor.matmul(wups[:], wu_w[:], wu_x[:],
                                     start=True, stop=True)

            filler(14)

            # DMA order: tile-0 weights + biases first (so tile-0's chain
            # and ACTs are never input-gated), then the remaining x chunks
            # back-to-back (inter-chunk PE gaps stay under the ~3.4us HAM
            # MID window), then the weight stream w1..w15 which the SDMA
            # keeps ~2.5us/tile ahead of the PE's ~3.7us/tile consumption.
            h = KT // 2
            XC = 8
            kk = KT // XC
            w0 = wpool.tile([P, KT, P], F32R, tag="w", bufs=6)
            nc.sync.dma_start(w0[:, :h, :], wsyn_d[:, 0, :h, :])
            nc.sync.dma_start(xsb[:, 0:kk, :], xh_d[:, 0:kk, :])
            nc.sync.dma_start(w0[:, h:, :], wsyn_d[:, 0, h:, :])
            nc.sync.dma_start(bsynr[:], bsynr_d[:])
            nc.sync.dma_start(bs100[:], bs100_d[:])
            nc.sync.dma_start(btm[:], btm_d[:])
            for c in range(1, XC):
                nc.sync.dma_start(xsb[:, c * kk:(c + 1) * kk, :],
                                  xh_d[:, c * kk:(c + 1) * kk, :])

            # ---- phase 1: L1 = W_syn @ x (transposed layout [o, batch])
            for t in range(OT):
                if t == 0:
                    w = w0
                else:
                    w = wpool.tile([P, KT, P], F32R, tag="w", bufs=6)
                    nc.sync.dma_start(w[:], wsyn_d[:, t])
                ps = psum1.tile([P, BC], F32, tag="ps1")
                if t == 0:
                    # chunk the chain and pad each x-wait with fillers
                    for c in range(XC):
                        for j in range(kk):
                            k = c * kk + j
                            nc.tensor.matmul(ps[:], w[:, k, :], xsb[:, k, :],
                                             start=(k == 0),
                                             stop=(k == KT - 1))
                        if c < XC - 1:
                            filler(3)
                else:
                    if t <= 3:
                        filler(4)
                    for k in range(KT):
                        nc.tensor.matmul(ps[:], w[:, k, :], xsb[:, k, :],
                                         start=(k == 0), stop=(k == KT - 1))
                nc.scalar.activation(l1r[:, t, :], ps[:], AF.Identity,
                                     bias=bsynr[:, t:t + 1], scale=a_scale)
                nc.scalar.activation(l1s[:, t, :], ps[:], AF.Identity,
                                     bias=bs100[:, t:t + 1], scale=100.0)

            # ---- phase 2: z1 = W_Tm @ L1; spike = (100*L1 > sigmoid(z1+b_Tm))
            for t in range(OT):
                wa = wpool.tile([P, KT, P], w2_dt, tag="wa", bufs=6)
                nc.sync.dma_start(wa[:], wtm_d[:, t])
                ps2 = psum2.tile([P, BC], F32, tag="ps2")
                if fp8:
                    for k2 in range(KT // 2):
                        nc.tensor.matmul(
                            ps2[:], wa[:, 2 * k2:2 * k2 + 2, :],
                            l1r[:, 2 * k2:2 * k2 + 2, :],
                            start=(k2 == 0), stop=(k2 == KT // 2 - 1),
                            perf_mode=mybir.MatmulPerfMode.DoubleRow)
                else:
                    for k in range(KT):
                        nc.tensor.matmul(ps2[:], wa[:, k, :], l1r[:, k, :],
                                         start=(k == 0), stop=(k == KT - 1))
                s = tmp.tile([P, BC], F32, tag="s")
                nc.scalar.activation(s[:], ps2[:], AF.Sigmoid,
                                     bias=btm[:, t:t + 1], scale=sig_scale)
                o = outp.tile([P, BC], U8, tag="o")
                nc.vector.tensor_tensor(o[:], l1s[:, t, :], s[:], ALU.is_gt)
                nc.sync.dma_start(out_d[t], o[:])

    nc.compile()
    return nc


def _pack_weight(w: np.ndarray) -> np.ndarray:
    # [O, I] -> [p, o_tile, k_tile, m] with w[t*128+m, k*128+p] at [p, t, k, m]
    return np.ascontiguousarray(w.reshape(OT, P, KT, P).transpose(3, 0, 2, 1))


def _pack_bias(v: np.ndarray) -> np.ndarray:
    return np.ascontiguousarray(v.reshape(OT, P).T)


def prepare_in_maps(x_t, u_t, b_t, spk, W_syn, b_syn, W_Tm, b_Tm, W_Tadp, b_Tadp):
    """Fast-path input packing (states assumed zero; W_Tadp/b_Tadp unused)."""
    fp8 = FAST_MM2 == "fp8"
    wsyn = _pack_weight(np.asarray(W_syn, np.float32))
    wtm_f = _pack_weight(np.asarray(W_Tm, np.float32))
    if fp8:
        wtm = np.ascontiguousarray(
            (wtm_f * W2_SCALE).astype(ml_dtypes.float8_e4m3))
    else:
        wtm = np.ascontiguousarray(wtm_f.astype(ml_dtypes.bfloat16))
    b_syn = np.asarray(b_syn, np.float32)
    bsynr = _pack_bias((A2_SCALE if fp8 else 1.0) * b_syn)
    bs100 = _pack_bias(100.0 * b_syn)
    btm = _pack_bias(np.asarray(b_Tm, np.float32))

    in_maps = []
    for c in range(NCORES):
        sl = slice(c * BC, (c + 1) * BC)
        xc = np.asarray(x_t[sl], np.float32)
        xp = np.ascontiguousarray(xc.reshape(BC, KT, P).transpose(2, 1, 0))
        in_maps.append({
            "xh": xp, "wsyn": wsyn, "wtm": wtm,
            "bsynr": bsynr, "bs100": bs100, "btm": btm,
        })
    return in_maps


def unpack_output(results) -> np.ndarray:
    # per-core out: [OT, P, BC] u8 -> [BC, O] f32; concat over cores -> [B, O]
    parts = [r["out"].transpose(2, 0, 1).reshape(BC, O).astype(np.float32)
             for r in results]
    return np.ascontiguousarray(np.concatenate(parts, axis=0))


_NC_FAST = None


def get_nc():
    global _NC_FAST
    if _NC_FAST is None:
        _NC_FAST = build_nc_fast()
    return _NC_FAST


def run_sharded(in_maps, trace=False, **kw):
    nc = get_nc()
    return run_bass_kernel_spmd(nc, in_maps, list(range(NCORES)), trace=trace, **kw)


# ---------------------------------------------------------------------------
# General path: original 3-matmul kernel, for nonzero state tensors.
# ---------------------------------------------------------------------------

def build_nc_general():
    nc = bacc.Bacc("TRN2", target_bir_lowering=False, debug=False)

    xh_d = nc.dram_tensor("xh", (P, KT, BC), F32R, kind="ExternalInput").ap()
    u_d = nc.dram_tensor("u", (OT, P, BC), BF16, kind="ExternalInput").ap()
    b_d = nc.dram_tensor("b", (OT, P, BC), BF16, kind="ExternalInput").ap()
    spk_d = nc.dram_tensor("spk", (OT, P, BC), BF16, kind="ExternalInput").ap()
    wsyn_d = nc.dram_tensor("wsyn", (P, OT, KT, P), F32R, kind="ExternalInput").ap()
    wtm_d = nc.dram_tensor("wtm", (P, OT, KT, P), F32R, kind="ExternalInput").ap()
    wtadp_d = nc.dram_tensor("wtadp", (P, OT, KT, P), F32R, kind="ExternalInput").ap()
    bsyn_d = nc.dram_tensor("bsyn", (P, OT), F32, kind="ExternalInput").ap()
    nbtm_d = nc.dram_tensor("nbtm", (P, OT), F32, kind="ExternalInput").ap()
    btadp_d = nc.dram_tensor("btadp", (P, OT), F32, kind="ExternalInput").ap()
    out_d = nc.dram_tensor("out", (OT, P, BC), U8, kind="ExternalOutput").ap()

    with tile.TileContext(nc) as tc:
        with (
            tc.tile_pool(name="persist", bufs=1) as persist,
            tc.tile_pool(name="wpool", bufs=4) as wpool,
            tc.tile_pool(name="iopool", bufs=6) as iopool,
            tc.tile_pool(name="tmp", bufs=12) as tmp,
            tc.tile_pool(name="outp", bufs=3) as outp,
            tc.tile_pool(name="psum1", bufs=2, space="PSUM") as psum1,
            tc.tile_pool(name="psum2", bufs=6, space="PSUM") as psum2,
        ):
            xsb = persist.tile([P, KT, BC], F32R, tag="xsb")
            l1sb = persist.tile([P, OT, BC], F32, tag="l1sb")
            z1sb = persist.tile([P, OT, BC], F32R, tag="z1sb")
            z2sb = persist.tile([P, OT, BC], F32R, tag="z2sb")
            bsyn = persist.tile([P, OT], F32, tag="bsyn")
            nbtm = persist.tile([P, OT], F32, tag="nbtm")
            btadp = persist.tile([P, OT], F32, tag="btadp")

            h = KT // 2
            w0 = wpool.tile([P, KT, P], F32R, tag="w")
            nc.sync.dma_start(w0[:, :h, :], wsyn_d[:, 0, :h, :])
            for k in range(KT):
                nc.sync.dma_start(xsb[:, k, :], xh_d[:, k, :])
            nc.sync.dma_start(w0[:, h:, :], wsyn_d[:, 0, h:, :])
            nc.sync.dma_start(bsyn[:], bsyn_d[:])
            nc.sync.dma_start(nbtm[:], nbtm_d[:])
            nc.sync.dma_start(btadp[:], btadp_d[:])

            # ---- phase 1: L1 = W_syn @ x, Z1 = L1+u, Z2 = L1+b
            for t in range(OT):
                if t == 0:
                    w = w0
                else:
                    w = wpool.tile([P, KT, P], F32R, tag="w")
                    nc.sync.dma_start(w[:, :h, :], wsyn_d[:, t, :h, :])
                    nc.sync.dma_start(w[:, h:, :], wsyn_d[:, t, h:, :])
                ps = psum1.tile([P, BC], F32)
                for k in range(KT):
                    nc.tensor.matmul(ps[:], w[:, k, :], xsb[:, k, :],
                                     start=(k == 0), stop=(k == KT - 1))
                nc.scalar.activation(l1sb[:, t, :], ps[:], AF.Identity,
                                     bias=bsyn[:, t:t + 1])
                ut = iopool.tile([P, BC], BF16, tag="io")
                nc.sync.dma_start(ut[:], u_d[t])
                bt = iopool.tile([P, BC], BF16, tag="io")
                nc.sync.dma_start(bt[:], b_d[t])
                nc.vector.tensor_add(z1sb[:, t, :], l1sb[:, t, :], ut[:])
                nc.vector.tensor_add(z2sb[:, t, :], l1sb[:, t, :], bt[:])

            # ---- phase 2: alpha/rho branches + fused pointwise tail
            for t in range(OT):
                wa = wpool.tile([P, KT, P], F32R, tag="w")
                nc.sync.dma_start(wa[:], wtm_d[:, t])
                wr = wpool.tile([P, KT, P], F32R, tag="w")
                nc.sync.dma_start(wr[:], wtadp_d[:, t])
                ut = iopool.tile([P, BC], BF16, tag="io")
                nc.sync.dma_start(ut[:], u_d[t])
                bt = iopool.tile([P, BC], BF16, tag="io")
                nc.sync.dma_start(bt[:], b_d[t])
                spt = iopool.tile([P, BC], BF16, tag="io")
                nc.sync.dma_start(spt[:], spk_d[t])

                l1t = l1sb[:, t, :]
                # u_new - thr = t1*e + (L1 - 1.8*spk) - 1.8*rho*(b-spk) - 0.01
                sp = tmp.tile([P, BC], F32, tag="t")
                nc.scalar.activation(sp[:], spt[:], AF.Copy, scale=-1.8)
                t1 = tmp.tile([P, BC], F32, tag="t")
                nc.vector.tensor_sub(t1[:], l1t, ut[:])
                t2 = tmp.tile([P, BC], F32, tag="t")
                nc.vector.tensor_sub(t2[:], bt[:], spt[:])
                s = tmp.tile([P, BC], F32, tag="t")
                nc.vector.tensor_add(s[:], l1t, sp[:])

                psa = psum2.tile([P, BC], F32, tag="ps2")
                for k in range(KT):
                    nc.tensor.matmul(psa[:], wa[:, k, :], z1sb[:, k, :],
                                     start=(k == 0), stop=(k == KT - 1))
                psr = psum2.tile([P, BC], F32, tag="ps2")
                for k in range(KT):
                    nc.tensor.matmul(psr[:], wr[:, k, :], z2sb[:, k, :],
                                     start=(k == 0), stop=(k == KT - 1))

                # e = exp(-(z1 + b_Tm)) = 1/alpha - 1; rho = sigmoid(z2 + b_Tadp)
                e = tmp.tile([P, BC], F32, tag="t")
                nc.scalar.activation(e[:], psa[:], AF.Exp,
                                     bias=nbtm[:, t:t + 1], scale=-1.0)
                rho = tmp.tile([P, BC], F32, tag="t")
                nc.scalar.activation(rho[:], psr[:], AF.Sigmoid,
                                     bias=btadp[:, t:t + 1])

                m = tmp.tile([P, BC], F32, tag="t")
                nc.vector.tensor_mul(m[:], t1[:], e[:])
                m2 = tmp.tile([P, BC], F32, tag="t")
                nc.vector.tensor_mul(m2[:], rho[:], t2[:])
                m2s = tmp.tile([P, BC], F32, tag="t")
                nc.scalar.activation(m2s[:], m2[:], AF.Copy, scale=1.8)
                d1 = tmp.tile([P, BC], F32, tag="t")
                nc.vector.tensor_add(d1[:], m[:], s[:])
                d = tmp.tile([P, BC], F32, tag="t")
                nc.vector.tensor_sub(d[:], d1[:], m2s[:])
                o = outp.tile([P, BC], U8, tag="o")
                nc.vector.tensor_scalar(o[:], d[:], THR_MIN, None, ALU.is_gt)
                nc.sync.dma_start(out_d[t], o[:])

    nc.compile()
    return nc


def _pack_state(v: np.ndarray) -> np.ndarray:
    return np.ascontiguousarray(
        v.reshape(BC, OT, P).transpose(1, 2, 0).astype(ml_dtypes.bfloat16))


def prepare_in_maps_general(x_t, u_t, b_t, spk, W_syn, b_syn, W_Tm, b_Tm,
                            W_Tadp, b_Tadp):
    wsyn = _pack_weight(np.asarray(W_syn, np.float32))
    wtm = _pack_weight(np.asarray(W_Tm, np.float32))
    wtadp = _pack_weight(np.asarray(W_Tadp, np.float32))
    bsyn = _pack_bias(np.asarray(b_syn, np.float32))
    nbtm = _pack_bias(-np.asarray(b_Tm, np.float32))
    btadp = _pack_bias(np.asarray(b_Tadp, np.float32))

    in_maps = []
    for c in range(NCORES):
        sl = slice(c * BC, (c + 1) * BC)
        xc = np.asarray(x_t[sl], np.float32)
        xp = np.ascontiguousarray(xc.reshape(BC, KT, P).transpose(2, 1, 0))
        in_maps.append({
            "xh": xp,
            "u": _pack_state(np.asarray(u_t[sl], np.float32)),
            "b": _pack_state(np.asarray(b_t[sl], np.float32)),
            "spk": _pack_state(np.asarray(spk[sl], np.float32)),
            "wsyn": wsyn, "wtm": wtm, "wtadp": wtadp,
            "bsyn": bsyn, "nbtm": nbtm, "btadp": btadp,
        })
    return in_maps


_NC_GENERAL = None


def get_nc_general():
    global _NC_GENERAL
    if _NC_GENERAL is None:
        _NC_GENERAL = build_nc_general()
    return _NC_GENERAL


def kernel(**inputs) -> np.ndarray:
    inputs = {k: np.asarray(v) for k, v in inputs.items()}
    states_zero = not (inputs["u_t"].any() or inputs["b_t"].any()
                       or inputs["spk"].any())
    if states_zero:
        in_maps = prepare_in_maps(**inputs)
        res = run_sharded(in_maps)
    else:
        in_maps = prepare_in_maps_general(**inputs)
        res = run_bass_kernel_spmd(get_nc_general(), in_maps,
                                   list(range(NCORES)))
    return unpack_output(res.results)


# revision 31
# speedup vs baseline: 1.0098x; 1.0098x over previous
"""LSNN layer forward on 8 Trainium2 NeuronCores (data-parallel over batch).

Reference math (per batch row):
    L1    = x_t @ W_syn.T + b_syn
    alpha = sigmoid((L1 + u_t) @ W_Tm.T + b_Tm)
    rho   = sigmoid((L1 + b_t) @ W_Tadp.T + b_Tadp)
    b_new = rho * b_t + (1 - rho) * spk
    thr   = 0.01 + 1.8 * b_new
    u_new = u_t + (L1 - u_t) / alpha
    o_spk = (u_new - thr > 0) as f32

Fast path (used when u_t == b_t == spk == 0, which the runtime check in
kernel() verifies): the rho branch cannot affect the output
(b_new = rho*0 + (1-rho)*0 = 0, thr = 0.01) and u_new = L1 / alpha with
alpha = sigmoid(L1 @ W_Tm.T + b_Tm) > 0, so

    o_spk = (L1 / alpha > 0.01) = (100 * L1 > sigmoid(z1 + b_Tm))

Only two device matmuls remain:
  phase 1: L1 = W_syn @ x in f32r (precision-critical; 1 PE col/cycle)
  phase 2: z1 = W_Tm @ L1 in float8e4 DoubleRow (weights x64, L1 x8 to
           stay in the fp8 normal range; 2 k-tiles per matmul = 2x PE
           throughput; the sigmoid rescales by 1/512 for free). The
           sigmoid path is heavily damped: fp8 moves only ~40 spikes
           out of 8.4M beyond the f32r-mm1 baseline of ~322.
Per-tile tail is one ACT sigmoid + one DVE is_gt compare (u8 out); only
the sigmoid table set is loaded (the baseline's per-tile Exp/Sigmoid
alternation cost 32 ACT table loads = 41us).
Dummy warm-up matmuls fire the PE HAM clock gate (1.2 -> 2.4 GHz) while
the x DMA head is still landing.

General path (any nonzero state): the original 3-matmul kernel, kept
verbatim below.

Sharding: batch 4096 -> 8 shards of 512; weights replicated; no
cross-core communication.

Measured on trn2 (8 cores, per-core 512x2048 batch shard):
    original baseline    227.4us, rel err 8.78e-3 (322/8.4M spike flips)
    fast bf16 mm2        145.0us, rel err 8.9e-3
    fast fp8-DR mm2      116.6us, rel err 9.34e-3 (364 flips)
Engine budget of the fast path per core: ~8.5us NEFF preamble/barriers,
~16us x+w0 DMA head (PE warms up + runs tile 0), ~52us phase-1 (256
f32r MMs, DMA stays ahead), ~27.6us phase-2 (128 fp8-DR MMs), ~5us
sigmoid/compare/output tail + postamble. DMA total 26MB at ~350-420GB/s
effective; PE is the bottleneck (target_regime=compute).
"""

import os

import numpy as np
import ml_dtypes

import concourse.bacc as bacc
import concourse.tile as tile
import concourse.mybir as mybir
from concourse.bass_utils import run_bass_kernel_spmd

AF = mybir.ActivationFunctionType
ALU = mybir.AluOpType

B, I, O = 4096, 2048, 2048
NCORES = 8
BC = B // NCORES          # 512 batch rows per core
P = 128                   # partitions
KT = I // P               # 16 k-tiles
OT = O // P               # 16 output neuron tiles
THR_MIN = 0.01

F32 = mybir.dt.float32
F32R = mybir.dt.float32r
BF16 = mybir.dt.bfloat16
FP8 = mybir.dt.float8e4
U8 = mybir.dt.uint8

# fast-path mm2 dtype: "fp8" uses float8e4 weights (x64) + activations
# (x8) with DoubleRow (2 k-tiles per matmul, 2x tensor-engine
# throughput); "bf16" casts L1 to bf16 (both operands bf16).
# ("mixed" bf16 x f32r is rejected by the BIR verifier: fp32/f32r operands
# must have matching transfer types.)
FAST_MM2 = os.environ.get("FAST_MM2", "fp8")
W2_SCALE = 64.0   # fp8 weight pre-scale (power of 2, exact)
A2_SCALE = 8.0    # fp8 activation pre-scale


def build_nc_fast():
    fp8 = FAST_MM2 == "fp8"
    rhs_dt = FP8 if fp8 else (F32R if FAST_MM2 == "mixed" else BF16)
    w2_dt = FP8 if fp8 else BF16
    # ACT scale folding: l1r holds A2_SCALE*L1 in fp8 mode; the sigmoid
    # rescales psum by 1/(W2_SCALE*A2_SCALE).
    a_scale = A2_SCALE if fp8 else 1.0
    sig_scale = 1.0 / (W2_SCALE * A2_SCALE) if fp8 else 1.0

    nc = bacc.Bacc("TRN2", target_bir_lowering=False, debug=False)

    xh_d = nc.dram_tensor("xh", (P, KT, BC), F32R, kind="ExternalInput").ap()
    wsyn_d = nc.dram_tensor("wsyn", (P, OT, KT, P), F32R, kind="ExternalInput").ap()
    wtm_d = nc.dram_tensor("wtm", (P, OT, KT, P), w2_dt, kind="ExternalInput").ap()
    bsynr_d = nc.dram_tensor("bsynr", (P, OT), F32, kind="ExternalInput").ap()
    bs100_d = nc.dram_tensor("bs100", (P, OT), F32, kind="ExternalInput").ap()
    btm_d = nc.dram_tensor("btm", (P, OT), F32, kind="ExternalInput").ap()
    out_d = nc.dram_tensor("out", (OT, P, BC), U8, kind="ExternalOutput").ap()

    with tile.TileContext(nc) as tc:
        with (
            tc.tile_pool(name="persist", bufs=1) as persist,
            tc.tile_pool(name="wpool", bufs=6) as wpool,
            tc.tile_pool(name="tmp", bufs=4) as tmp,
            tc.tile_pool(name="outp", bufs=3) as outp,
            tc.tile_pool(name="psum1", bufs=2, space="PSUM") as psum1,
            tc.tile_pool(name="psum2", bufs=5, space="PSUM") as psum2,
            tc.tile_pool(name="psumw", bufs=1, space="PSUM") as psumw,
        ):
            xsb = persist.tile([P, KT, BC], F32R, tag="xsb")
            l1r = persist.tile([P, OT, BC], rhs_dt, tag="l1r")   # a_scale*L1
            l1s = persist.tile([P, OT, BC], F32, tag="l1s")      # 100*L1
            bsynr = persist.tile([P, OT], F32, tag="bsynr")
            bs100 = persist.tile([P, OT], F32, tag="bs100")
            btm = persist.tile([P, OT], F32, tag="btm")

            # PE warm-up: the HAM clock gate keeps the PE at 1.2 GHz until
            # it has seen ~3.4us of sustained matmul activity. The input
            # DMA head (x + first weight tiles) takes ~12us, during which
            # the PE would otherwise idle/dribble and stay cold. Dummy
            # matmuls on zeroed tiles into a scratch PSUM bank warm the
            # clock to 2.4 GHz before the real matmuls begin.
            wu_w = persist.tile([P, P], BF16, tag="wu_w")
            wu_x = persist.tile([P, BC], BF16, tag="wu_x")
            nc.vector.memset(wu_w[:], 0.0)
            nc.vector.memset(wu_x[:], 0.0)
            wups = psumw.tile([P, BC], F32, tag="wu_ps")

            def filler(n):
                # Dummy matmuls into the scratch PSUM bank. Interleaved
                # into the DMA-gated head they keep the PE's HAM activity
                # window busy, so the clock never drops back to 1.2 GHz
                # while x/weight chunks are still landing.
                for _ in range(n):
                    nc.tensor.matmul(wups[:], wu_w[:], wu_x[:],
                                     start=True, stop=True)

            filler(14)

            # DMA order: tile-0 weights + biases first (so tile-0's chain
            # and ACTs are never input-gated), then the remaining x chunks
            # back-to-back (inter-chunk PE gaps stay under the ~3.4us HAM
            # MID window), then the weight stream w1..w15 which the SDMA
            # keeps ~2.5us/tile ahead of the PE's ~3.7us/tile consumption.
            h = KT // 2
            XC = 8
            kk = KT // XC
            w0 = wpool.tile([P, KT, P], F32R, tag="w", bufs=6)
            nc.sync.dma_start(w0[:, :h, :], wsyn_d[:, 0, :h, :])
            nc.sync.dma_start(xsb[:, 0:kk, :], xh_d[:, 0:kk, :])
            nc.sync.dma_start(w0[:, h:, :], wsyn_d[:, 0, h:, :])
            nc.sync.dma_start(bsynr[:], bsynr_d[:])
            nc.sync.dma_start(bs100[:], bs100_d[:])
            nc.sync.dma_start(btm[:], btm_d[:])
            for c in range(1, XC):
                nc.sync.dma_start(xsb[:, c * kk:(c + 1) * kk, :],
                                  xh_d[:, c * kk:(c + 1) * kk, :])

            # ---- phase 1: L1 = W_syn @ x (transposed layout [o, batch])
            for t in range(OT):
                if t == 0:
                    w = w0
                else:
                    w = wpool.tile([P, KT, P], F32R, tag="w", bufs=6)
                    nc.sync.dma_start(w[:], wsyn_d[:, t])
                ps = psum1.tile([P, BC], F32, tag="ps1")
                if t == 0:
                    # chunk the chain and pad each x-wait with fillers
                    for c in range(XC):
                        for j in range(kk):
                            k = c * kk + j
                            nc.tensor.matmul(ps[:], w[:, k, :], xsb[:, k, :],
                                             start=(k == 0),
                                             stop=(k == KT - 1))
                        if c < XC - 1:
                            filler(3)
                else:
                    if t <= 4:
                        filler(8)
                    for k in range(KT):
                        nc.tensor.matmul(ps[:], w[:, k, :], xsb[:, k, :],
                                         start=(k == 0), stop=(k == KT - 1))
                nc.scalar.activation(l1r[:, t, :], ps[:], AF.Identity,
                                     bias=bsynr[:, t:t + 1], scale=a_scale)
                nc.scalar.activation(l1s[:, t, :], ps[:], AF.Identity,
                                     bias=bs100[:, t:t + 1], scale=100.0)

            # ---- phase 2: z1 = W_Tm @ L1; spike = (100*L1 > sigmoid(z1+b_Tm))
            for t in range(OT):
                wa = wpool.tile([P, KT, P], w2_dt, tag="wa", bufs=6)
                nc.sync.dma_start(wa[:], wtm_d[:, t])
                ps2 = psum2.tile([P, BC], F32, tag="ps2")
                if fp8:
                    for k2 in range(KT // 2):
                        nc.tensor.matmul(
                            ps2[:], wa[:, 2 * k2:2 * k2 + 2, :],
                            l1r[:, 2 * k2:2 * k2 + 2, :],
                            start=(k2 == 0), stop=(k2 == KT // 2 - 1),
                            perf_mode=mybir.MatmulPerfMode.DoubleRow)
                else:
                    for k in range(KT):
                        nc.tensor.matmul(ps2[:], wa[:, k, :], l1r[:, k, :],
                                         start=(k == 0), stop=(k == KT - 1))
                s = tmp.tile([P, BC], F32, tag="s")
                nc.scalar.activation(s[:], ps2[:], AF.Sigmoid,
                                     bias=btm[:, t:t + 1], scale=sig_scale)
                o = outp.tile([P, BC], U8, tag="o")
                nc.vector.tensor_tensor(o[:], l1s[:, t, :], s[:], ALU.is_gt)
                nc.sync.dma_start(out_d[t], o[:])

    nc.compile()
    return nc


def _pack_weight(w: np.ndarray) -> np.ndarray:
    # [O, I] -> [p, o_tile, k_tile, m] with w[t*128+m, k*128+p] at [p, t, k, m]
    return np.ascontiguousarray(w.reshape(OT, P, KT, P).transpose(3, 0, 2, 1))


def _pack_bias(v: np.ndarray) -> np.ndarray:
    return np.ascontiguousarray(v.reshape(OT, P).T)


def prepare_in_maps(x_t, u_t, b_t, spk, W_syn, b_syn, W_Tm, b_Tm, W_Tadp, b_Tadp):
    """Fast-path input packing (states assumed zero; W_Tadp/b_Tadp unused)."""
    fp8 = FAST_MM2 == "fp8"
    wsyn = _pack_weight(np.asarray(W_syn, np.float32))
    wtm_f = _pack_weight(np.asarray(W_Tm, np.float32))
    if fp8:
        wtm = np.ascontiguousarray(
            (wtm_f * W2_SCALE).astype(ml_dtypes.float8_e4m3))
    else:
        wtm = np.ascontiguousarray(wtm_f.astype(ml_dtypes.bfloat16))
    b_syn = np.asarray(b_syn, np.float32)
    bsynr = _pack_bias((A2_SCALE if fp8 else 1.0) * b_syn)
    bs100 = _pack_bias(100.0 * b_syn)
    btm = _pack_bias(np.asarray(b_Tm, np.float32))

    in_maps = []
    for c in range(NCORES):
        sl = slice(c * BC, (c + 1) * BC)
        xc = np.asarray(x_t[sl], np.float32)
        xp = np.ascontiguousarray(xc.reshape(BC, KT, P).transpose(2, 1, 0))
        in_maps.append({
            "xh": xp, "wsyn": wsyn, "wtm": wtm,
            "bsynr": bsynr, "bs100": bs100, "btm": btm,
        })
    return in_maps


def unpack_output(results) -> np.ndarray:
    # per-core out: [OT, P, BC] u8 -> [BC, O] f32; concat over cores -> [B, O]
    parts = [r["out"].transpose(2, 0, 1).reshape(BC, O).astype(np.float32)
             for r in results]
    return np.ascontiguousarray(np.concatenate(parts, axis=0))


_NC_FAST = None


def get_nc():
    global _NC_FAST
    if _NC_FAST is None:
        _NC_FAST = build_nc_fast()
    return _NC_FAST


def run_sharded(in_maps, trace=False, **kw):
    nc = get_nc()
    return run_bass_kernel_spmd(nc, in_maps, list(range(NCORES)), trace=trace, **kw)


# ---------------------------------------------------------------------------
# General path: original 3-matmul kernel, for nonzero state tensors.
# ---------------------------------------------------------------------------

def build_nc_general():
    nc = bacc.Bacc("TRN2", target_bir_lowering=False, debug=False)

    xh_d = nc.dram_tensor("xh", (P, KT, BC), F32R, kind="ExternalInput").ap()
    u_d = nc.dram_tensor("u", (OT, P, BC), BF16, kind="ExternalInput").ap()
    b_d = nc.dram_tensor("b", (OT, P, BC), BF16, kind="ExternalInput").ap()
    spk_d = nc.dram_tensor("spk", (OT, P, BC), BF16, kind="ExternalInput").ap()
    wsyn_d = nc.dram_tensor("wsyn", (P, OT, KT, P), F32R, kind="ExternalInput").ap()
    wtm_d = nc.dram_tensor("wtm", (P, OT, KT, P), F32R, kind="ExternalInput").ap()
    wtadp_d = nc.dram_tensor("wtadp", (P, OT, KT, P), F32R, kind="ExternalInput").ap()
    bsyn_d = nc.dram_tensor("bsyn", (P, OT), F32, kind="ExternalInput").ap()
    nbtm_d = nc.dram_tensor("nbtm", (P, OT), F32, kind="ExternalInput").ap()
    btadp_d = nc.dram_tensor("btadp", (P, OT), F32, kind="ExternalInput").ap()
    out_d = nc.dram_tensor("out", (OT, P, BC), U8, kind="ExternalOutput").ap()

    with tile.TileContext(nc) as tc:
        with (
            tc.tile_pool(name="persist", bufs=1) as persist,
            tc.tile_pool(name="wpool", bufs=4) as wpool,
            tc.tile_pool(name="iopool", bufs=6) as iopool,
            tc.tile_pool(name="tmp", bufs=12) as tmp,
            tc.tile_pool(name="outp", bufs=3) as outp,
            tc.tile_pool(name="psum1", bufs=2, space="PSUM") as psum1,
            tc.tile_pool(name="psum2", bufs=6, space="PSUM") as psum2,
        ):
            xsb = persist.tile([P, KT, BC], F32R, tag="xsb")
            l1sb = persist.tile([P, OT, BC], F32, tag="l1sb")
            z1sb = persist.tile([P, OT, BC], F32R, tag="z1sb")
            z2sb = persist.tile([P, OT, BC], F32R, tag="z2sb")
            bsyn = persist.tile([P, OT], F32, tag="bsyn")
            nbtm = persist.tile([P, OT], F32, tag="nbtm")
            btadp = persist.tile([P, OT], F32, tag="btadp")

            h = KT // 2
            w0 = wpool.tile([P, KT, P], F32R, tag="w")
            nc.sync.dma_start(w0[:, :h, :], wsyn_d[:, 0, :h, :])
            for k in range(KT):
                nc.sync.dma_start(xsb[:, k, :], xh_d[:, k, :])
            nc.sync.dma_start(w0[:, h:, :], wsyn_d[:, 0, h:, :])
            nc.sync.dma_start(bsyn[:], bsyn_d[:])
            nc.sync.dma_start(nbtm[:], nbtm_d[:])
            nc.sync.dma_start(btadp[:], btadp_d[:])

            # ---- phase 1: L1 = W_syn @ x, Z1 = L1+u, Z2 = L1+b
            for t in range(OT):
                if t == 0:
                    w = w0
                else:
                    w = wpool.tile([P, KT, P], F32R, tag="w")
                    nc.sync.dma_start(w[:, :h, :], wsyn_d[:, t, :h, :])
                    nc.sync.dma_start(w[:, h:, :], wsyn_d[:, t, h:, :])
                ps = psum1.tile([P, BC], F32)
                for k in range(KT):
                    nc.tensor.matmul(ps[:], w[:, k, :], xsb[:, k, :],
                                     start=(k == 0), stop=(k == KT - 1))
                nc.scalar.activation(l1sb[:, t, :], ps[:], AF.Identity,
                                     bias=bsyn[:, t:t + 1])
                ut = iopool.tile([P, BC], BF16, tag="io")
                nc.sync.dma_start(ut[:], u_d[t])
                bt = iopool.tile([P, BC], BF16, tag="io")
                nc.sync.dma_start(bt[:], b_d[t])
                nc.vector.tensor_add(z1sb[:, t, :], l1sb[:, t, :], ut[:])
                nc.vector.tensor_add(z2sb[:, t, :], l1sb[:, t, :], bt[:])

            # ---- phase 2: alpha/rho branches + fused pointwise tail
            for t in range(OT):
                wa = wpool.tile([P, KT, P], F32R, tag="w")
                nc.sync.dma_start(wa[:], wtm_d[:, t])
                wr = wpool.tile([P, KT, P], F32R, tag="w")
                nc.sync.dma_start(wr[:], wtadp_d[:, t])
                ut = iopool.tile([P, BC], BF16, tag="io")
                nc.sync.dma_start(ut[:], u_d[t])
                bt = iopool.tile([P, BC], BF16, tag="io")
                nc.sync.dma_start(bt[:], b_d[t])
                spt = iopool.tile([P, BC], BF16, tag="io")
                nc.sync.dma_start(spt[:], spk_d[t])

                l1t = l1sb[:, t, :]
                # u_new - thr = t1*e + (L1 - 1.8*spk) - 1.8*rho*(b-spk) - 0.01
                sp = tmp.tile([P, BC], F32, tag="t")
                nc.scalar.activation(sp[:], spt[:], AF.Copy, scale=-1.8)
                t1 = tmp.tile([P, BC], F32, tag="t")
                nc.vector.tensor_sub(t1[:], l1t, ut[:])
                t2 = tmp.tile([P, BC], F32, tag="t")
                nc.vector.tensor_sub(t2[:], bt[:], spt[:])
                s = tmp.tile([P, BC], F32, tag="t")
                nc.vector.tensor_add(s[:], l1t, sp[:])

                psa = psum2.tile([P, BC], F32, tag="ps2")
                for k in range(KT):
                    nc.tensor.matmul(psa[:], wa[:, k, :], z1sb[:, k, :],
                                     start=(k == 0), stop=(k == KT - 1))
                psr = psum2.tile([P, BC], F32, tag="ps2")
                for k in range(KT):
                    nc.tensor.matmul(psr[:], wr[:, k, :], z2sb[:, k, :],
                                     start=(k == 0), stop=(k == KT - 1))

                # e = exp(-(z1 + b_Tm)) = 1/alpha - 1; rho = sigmoid(z2 + b_Tadp)
                e = tmp.tile([P, BC], F32, tag="t")
                nc.scalar.activation(e[:], psa[:], AF.Exp,
                                     bias=nbtm[:, t:t + 1], scale=-1.0)
                rho = tmp.tile([P, BC], F32, tag="t")
                nc.scalar.activation(rho[:], psr[:], AF.Sigmoid,
                                     bias=btadp[:, t:t + 1])

                m = tmp.tile([P, BC], F32, tag="t")
                nc.vector.tensor_mul(m[:], t1[:], e[:])
                m2 = tmp.tile([P, BC], F32, tag="t")
                nc.vector.tensor_mul(m2[:], rho[:], t2[:])
                m2s = tmp.tile([P, BC], F32, tag="t")
                nc.scalar.activation(m2s[:], m2[:], AF.Copy, scale=1.8)
                d1 = tmp.tile([P, BC], F32, tag="t")
                nc.vector.tensor_add(d1[:], m[:], s[:])
                d = tmp.tile([P, BC], F32, tag="t")
                nc.vector.tensor_sub(d[:], d1[:], m2s[:])
                o = outp.tile([P, BC], U8, tag="o")
                nc.vector.tensor_scalar(o[:], d[:], THR_MIN, None, ALU.is_gt)
                nc.sync.dma_start(out_d[t], o[:])

    nc.compile()
    return nc


def _pack_state(v: np.ndarray) -> np.ndarray:
    return np.ascontiguousarray(
        v.reshape(BC, OT, P).transpose(1, 2, 0).astype(ml_dtypes.bfloat16))


def prepare_in_maps_general(x_t, u_t, b_t, spk, W_syn, b_syn, W_Tm, b_Tm,
                            W_Tadp, b_Tadp):
    wsyn = _pack_weight(np.asarray(W_syn, np.float32))
    wtm = _pack_weight(np.asarray(W_Tm, np.float32))
    wtadp = _pack_weight(np.asarray(W_Tadp, np.float32))
    bsyn = _pack_bias(np.asarray(b_syn, np.float32))
    nbtm = _pack_bias(-np.asarray(b_Tm, np.float32))
    btadp = _pack_bias(np.asarray(b_Tadp, np.float32))

    in_maps = []
    for c in range(NCORES):
        sl = slice(c * BC, (c + 1) * BC)
        xc = np.asarray(x_t[sl], np.float32)
        xp = np.ascontiguousarray(xc.reshape(BC, KT, P).transpose(2, 1, 0))
        in_maps.append({
            "xh": xp,
            "u": _pack_state(np.asarray(u_t[sl], np.float32)),
            "b": _pack_state(np.asarray(b_t[sl], np.float32)),
            "spk": _pack_state(np.asarray(spk[sl], np.float32)),
            "wsyn": wsyn, "wtm": wtm, "wtadp": wtadp,
            "bsyn": bsyn, "nbtm": nbtm, "btadp": btadp,
        })
    return in_maps


_NC_GENERAL = None


def get_nc_general():
    global _NC_GENERAL
    if _NC_GENERAL is None:
        _NC_GENERAL = build_nc_general()
    return _NC_GENERAL


def kernel(**inputs) -> np.ndarray:
    inputs = {k: np.asarray(v) for k, v in inputs.items()}
    states_zero = not (inputs["u_t"].any() or inputs["b_t"].any()
                       or inputs["spk"].any())
    if states_zero:
        in_maps = prepare_in_maps(**inputs)
        res = run_sharded(in_maps)
    else:
        in_maps = prepare_in_maps_general(**inputs)
        res = run_bass_kernel_spmd(get_nc_general(), in_maps,
                                   list(range(NCORES)))
    return unpack_output(res.results)
